# revision 1
# baseline (speedup 1.0000x reference)
"""Multi-head attention (B=4, S=2048, D=1024, H=16) on 8 trn2 cores.

Sharding: core c -> batch b = c//2, head-half = c%2 (8 heads = 512 dims).
Each core computes attention for its (batch, 8 heads) and a partial output
projection over its 512 d-features; the host sums the two partials per batch
and adds the (bo + bv @ Wo.T) constant row vector.

Device dataflow (per core, all shapes hardcoded):
  Phase A: QK^T projections into transposed layout Q^T/K^T [128d(2 heads), S]
           (bf16), V in [s, d] layout (f32r) with key-padding mask folded in
           and a mask column appended per head ([V'|m], 65 cols/head).
  Phase B: per head: S^T[k,q] = K^T.T @ Q^T tiles -> ACT exp(x/8) -> P^T
           (f32r); [num^T; denom] = [V'|m].T @ P^T accumulated over k-tiles;
           reciprocal of denom row, PE ones-broadcast, DVE multiply ->
           valsT [d, s] (f32r).
  Phase C: out[s,:] += valsT.T @ WoT accumulated over the 4 head-pair blocks.
"""

import numpy as np
from contextlib import ExitStack

import concourse.bacc as bacc
import concourse.tile as tile
import concourse.mybir as mybir
from concourse.bass_utils import run_bass_kernel_spmd

F32 = mybir.dt.float32
F32R = mybir.dt.float32r
BF16 = mybir.dt.bfloat16
EXP = mybir.ActivationFunctionType.Exp

S = 2048          # sequence length
D = 1024          # model dim
HD = 64           # head dim
NHL = 8           # heads per core
HP = 4            # head pairs per core (128 dims each)
DLOC = 512        # d-features per core
ET = D // 128     # 8 contraction tiles over D
ST = S // 128     # 16 s-tiles
QC = S // 512     # 4 query chunks of 512
KK = S // 128     # 16 key tiles of 128
VW = HD + 1       # V block width per head incl. mask column


def build_nc():
    nc = bacc.Bacc(None)
    xT = nc.dram_tensor("xT", [D, S], F32, kind="ExternalInput")
    wqT = nc.dram_tensor("wqT", [D, DLOC], F32, kind="ExternalInput")
    wkT = nc.dram_tensor("wkT", [D, DLOC], F32, kind="ExternalInput")
    wvT = nc.dram_tensor("wvT", [D, DLOC], F32, kind="ExternalInput")
    woT = nc.dram_tensor("woT", [DLOC, D], F32, kind="ExternalInput")
    bq = nc.dram_tensor("bq", [DLOC, 1], F32, kind="ExternalInput")
    bk = nc.dram_tensor("bk", [DLOC, 1], F32, kind="ExternalInput")
    msk = nc.dram_tensor("msk", [S, 1], F32, kind="ExternalInput")
    out = nc.dram_tensor("out", [S, D], F32, kind="ExternalOutput")

    with tile.TileContext(nc) as tc, ExitStack() as ctx:
        res = ctx.enter_context(tc.tile_pool(name="res", bufs=1))

        kt = [res.tile([128, S], F32R, tag=f"kt{i}", name=f"kt{i}") for i in range(HP)]
        vm = [res.tile([128, NHL * VW], F32R, tag=f"vm{i}", name=f"vm{i}") for i in range(KK)]
        valsT = [res.tile([128, S], F32R, tag=f"valsT{i}", name=f"valsT{i}") for i in range(HP)]
        # current-chunk Q^T tiles, rewritten every q-chunk (double-buffered)
        qtp = ctx.enter_context(tc.tile_pool(name="qtp", bufs=2))

        m_sb = res.tile([128, ST], F32, tag="m_sb")
        nc.sync.dma_start(out=m_sb, in_=msk.rearrange("(a p) o -> p (a o)", p=128))
        bq_sb = res.tile([128, HP], F32, tag="bq_sb")
        nc.sync.dma_start(out=bq_sb, in_=bq.rearrange("(a p) o -> p (a o)", p=128))
        bk_sb = res.tile([128, HP], F32, tag="bk_sb")
        nc.sync.dma_start(out=bk_sb, in_=bk.rearrange("(a p) o -> p (a o)", p=128))

        # Q / O weights prefetched during pass 1
        pb = ctx.enter_context(tc.tile_pool(name="pb", bufs=1))
        wq_sb = [pb.tile([128, DLOC], F32R, tag=f"wq{e}", name=f"wq{e}") for e in range(ET)]
        wo_sb = [pb.tile([128, D], F32R, tag=f"wo{i}", name=f"wo{i}") for i in range(HP)]

        # ---------- Pass 1: K and V projections (stream xT once) ----------
        with tc.tile_pool(name="pa", bufs=1) as pa, \
             tc.tile_pool(name="xtp", bufs=10) as xtp, \
             tc.tile_pool(name="psA", bufs=4, space="PSUM") as psA:
            wk_sb = [pa.tile([128, DLOC], F32R, tag=f"wk{e}", name=f"wk{e}") for e in range(ET)]
            wv_sb = [pa.tile([128, DLOC], F32R, tag=f"wv{e}", name=f"wv{e}") for e in range(ET)]
            for e in range(ET):
                nc.sync.dma_start(out=wk_sb[e],
                                  in_=wkT[e * 128:(e + 1) * 128, :].bitcast(F32R))
            for qc in range(QC):
                cs = slice(qc * 512, (qc + 1) * 512)
                xts = [xtp.tile([128, 512], F32R, tag="xt", name=f"xt{qc}_{e}") for e in range(ET)]
                for e in range(ET):
                    nc.sync.dma_start(out=xts[e],
                                      in_=xT[e * 128:(e + 1) * 128, cs].bitcast(F32R))
                if qc == 0:
                    for e in range(ET):
                        nc.sync.dma_start(out=wv_sb[e],
                                          in_=wvT[e * 128:(e + 1) * 128, :].bitcast(F32R))
                elif qc == 1:
                    for e in range(ET):
                        nc.sync.dma_start(out=wq_sb[e],
                                          in_=wqT[e * 128:(e + 1) * 128, :].bitcast(F32R))
                    for i in range(HP):
                        nc.sync.dma_start(out=wo_sb[i],
                                          in_=woT[i * 128:(i + 1) * 128, :].bitcast(F32R))
                for hp in range(HP):
                    hcols = slice(hp * 128, (hp + 1) * 128)
                    psK = psA.tile([128, 512], F32, tag="psA")
                    for e in range(ET):
                        nc.tensor.matmul(psK, wk_sb[e][:, hcols], xts[e],
                                         start=(e == 0), stop=(e == ET - 1))
                    nc.vector.tensor_scalar_add(kt[hp][:, cs], psK, bk_sb[:, hp:hp + 1])
                for j in range(4):
                    st = qc * 4 + j
                    js = slice(j * 128, (j + 1) * 128)
                    psV = psA.tile([128, 512], F32, tag="psA")
                    for e in range(ET):
                        nc.tensor.matmul(psV, xts[e][:, js], wv_sb[e],
                                         start=(e == 0), stop=(e == ET - 1))
                    mc = m_sb[:, st:st + 1]
                    for h in range(NHL):
                        nc.vector.tensor_scalar_mul(
                            vm[st][:, h * VW:h * VW + HD],
                            psV[:, h * HD:(h + 1) * HD], mc)
                        nc.gpsimd.tensor_copy(out=vm[st][:, h * VW + HD:h * VW + VW], in_=mc)

        # ---- Pass 2: per q-chunk: Q projection + attention + out proj ----
        with tc.tile_pool(name="xtq", bufs=8) as xtq, \
             tc.tile_pool(name="ptp", bufs=7) as ptp, \
             tc.tile_pool(name="sm", bufs=2) as sm, \
             tc.tile_pool(name="ob", bufs=2) as ob, \
             tc.tile_pool(name="psS", bufs=3, space="PSUM") as psSp, \
             tc.tile_pool(name="psO", bufs=2, space="PSUM") as psOp:
            def emit_qproj(qc):
                cs_q = slice(qc * 512, (qc + 1) * 512)
                xts = [xtq.tile([128, 512], F32R, tag="xt2", name=f"x2{qc}_{e}") for e in range(ET)]
                for e in range(ET):
                    nc.sync.dma_start(out=xts[e],
                                      in_=xT[e * 128:(e + 1) * 128, cs_q].bitcast(F32R))
                qtc = [qtp.tile([128, 512], F32R, tag=f"qt{hp}", name=f"qt{qc}_{hp}")
                       for hp in range(HP)]
                for hp in range(HP):
                    hcols = slice(hp * 128, (hp + 1) * 128)
                    psQ = psSp.tile([128, 512], F32, tag="psS")
                    for e in range(ET):
                        nc.tensor.matmul(psQ, wq_sb[e][:, hcols], xts[e],
                                         start=(e == 0), stop=(e == ET - 1))
                    nc.vector.tensor_scalar_add(qtc[hp], psQ, bq_sb[:, hp:hp + 1])
                return qtc

            qt_next = emit_qproj(0)
            for qc in range(QC):
                cs = slice(qc * 512, (qc + 1) * 512)
                qtc = qt_next

                for hp in range(HP):
                    pts = [[], []]
                    for kp in range(KK // 2):
                        psS = [None, None]
                        for h2 in range(2):
                            psS[h2] = psSp.tile([128, 1024], F32, tag="psS",
                                                name=f"psS{qc}_{hp}_{kp}_{h2}")
                        for u in range(2):
                            kk = kp * 2 + u
                            ks = slice(kk * 128, (kk + 1) * 128)
                            for h2 in range(2):
                                hr = slice(h2 * 64, (h2 + 1) * 64)
                                nc.tensor.matmul(
                                    psS[h2][:, u * 512:(u + 1) * 512],
                                    kt[hp][hr, ks], qtc[hp][hr, :],
                                    start=True, stop=True)
                        for h2 in range(2):
                            pt = ptp.tile([128, 1024], F32R, tag="pt",
                                          name=f"pt{qc}_{hp}_{kp}_{h2}")
                            nc.scalar.activation(pt, psS[h2], EXP, scale=0.125)
                            pts[h2].append(pt)
                    for h2 in range(2):
                        h = hp * 2 + h2
                        hr = slice(h2 * 64, (h2 + 1) * 64)
                        psO = psOp.tile([128, 512], F32, tag="psO",
                                        name=f"psO{qc}_{hp}_{h2}")
                        for kp in range(KK // 2):
                            for u in range(2):
                                kk = kp * 2 + u
                                nc.tensor.matmul(
                                    psO[0:VW, :],
                                    vm[kk][:, h * VW:(h + 1) * VW],
                                    pts[h2][kp][:, u * 512:(u + 1) * 512],
                                    start=(kk == 0), stop=(kk == KK - 1))
                        dn = sm.tile([1, 512], F32, tag="dn")
                        nc.vector.tensor_copy(dn, psO[HD:VW, :])
                        dnb = sm.tile([64, 512], F32, tag="dnb")
                        nc.gpsimd.partition_broadcast(dnb, dn)
                        nc.vector.reciprocal_approx_fast(out=dnb, in_=dnb)
                        nc.vector.tensor_mul(valsT[hp][hr, cs], psO[0:HD, :], dnb)

                if qc + 1 < QC:
                    qt_next = emit_qproj(qc + 1)

                for j in range(4):
                    st = qc * 4 + j
                    ss = slice(st * 128, (st + 1) * 128)
                    ot = ob.tile([128, D], F32, tag="ot", name=f"ot{st}")
                    for ec in range(2):
                        es = slice(ec * 512, (ec + 1) * 512)
                        psC = psOp.tile([128, 512], F32, tag="psO",
                                        name=f"psC{st}_{ec}")
                        for hp in range(HP):
                            nc.tensor.matmul(psC, valsT[hp][:, ss],
                                             wo_sb[hp][:, es],
                                             start=(hp == 0), stop=(hp == HP - 1))
                        nc.vector.tensor_copy(ot[:, es], psC)
                    nc.sync.dma_start(out=out[ss, :], in_=ot)

    nc.finalize()
    return nc


_NC_CACHE = None


def _get_nc():
    global _NC_CACHE
    if _NC_CACHE is None:
        _NC_CACHE = build_nc()
    return _NC_CACHE


def make_in_maps(x, mask, Wq, bq, Wk, bk, Wv, Wo):
    in_maps = []
    for c in range(8):
        b = c // 2
        dsl = slice((c % 2) * DLOC, (c % 2) * DLOC + DLOC)
        in_maps.append({
            "xT": np.ascontiguousarray(x[b].T, dtype=np.float32),
            "wqT": np.ascontiguousarray(Wq[dsl, :].T, dtype=np.float32),
            "wkT": np.ascontiguousarray(Wk[dsl, :].T, dtype=np.float32),
            "wvT": np.ascontiguousarray(Wv[dsl, :].T, dtype=np.float32),
            "woT": np.ascontiguousarray(Wo[:, dsl].T, dtype=np.float32),
            "bq": np.ascontiguousarray(bq[dsl], dtype=np.float32)[:, None],
            "bk": np.ascontiguousarray(bk[dsl], dtype=np.float32)[:, None],
            "msk": mask[b].astype(np.float32)[:, None],
        })
    return in_maps


def assemble(results, Wo, bo, bv):
    out = np.empty((4, S, D), dtype=np.float32)
    for b in range(4):
        out[b] = results[2 * b]["out"] + results[2 * b + 1]["out"]
    out += (bo + bv @ Wo.T).astype(np.float32)
    return out


def run(x, mask, Wq, bq, Wk, bk, Wv, bv, Wo, bo, trace=False):
    nc = _get_nc()
    in_maps = make_in_maps(x, mask, Wq, bq, Wk, bk, Wv, Wo)
    res = run_bass_kernel_spmd(nc, in_maps, list(range(8)), trace=trace)
    return assemble(res.results, Wo, bo, bv), res


def kernel(x, mask, Wq, bq, Wk, bk, Wv, bv, Wo, bo):
    out, _ = run(x, mask, Wq, bq, Wk, bk, Wv, bv, Wo, bo)
    return out



# revision 3
# speedup vs baseline: 1.3158x; 1.3158x over previous
"""Multi-head attention (B=4, S=2048, D=1024, H=16) on 8 trn2 cores.

Sharding: core c -> batch b = c//2, head-half = c%2 (8 heads = 512 dims).
Each core computes attention for its (batch, 8 heads) and a partial output
projection over its 512 d-features; the host sums the two partials per batch
and adds the (bo + bv @ Wo.T) constant row vector.

v2: all matmul operands in bf16 (f32 PSUM accumulation) so LDWEIGHTS uses
fast-weight-load and the PE stays dense at full clock; x is converted once
and kept resident in SBUF; the attention pass software-pipelines
scores(u) / exp(u) / PV(u-1) / Q-proj(qc+1) / out-proj(qc-1) at 2-matmul
granularity so the Scalar engine's exp stream (the throughput floor) always
has PSUM tiles ready and the PE never idles long enough to trip the HAM
clock throttle.

Device dataflow (per core, all shapes hardcoded):
  Pass 1: K^T [128d(2 heads), S] (bf16, bias folded), V' per k-tile
          [128s, 8*(64+1)] (bf16, key-padding mask folded, +mask column for
          the softmax denominator). x converted to bf16 (resident), weights
          converted on the Scalar engine.
  Pass 2: per (q-chunk 512, head-pair): S^T[k,q] = K^T.T @ Q^T tiles ->
          ACT exp(x/8) -> P^T (bf16); [num^T; denom] = [V'|m].T @ P^T
          accumulated over k-tiles; reciprocal+broadcast+multiply ->
          valsT [d, s] (bf16). out[s,:] += valsT.T @ WoT per s-tile.
"""

import numpy as np
from contextlib import ExitStack

import concourse.bacc as bacc
import concourse.tile as tile
import concourse.mybir as mybir
from concourse.bass_utils import run_bass_kernel_spmd

F32 = mybir.dt.float32
BF16 = mybir.dt.bfloat16
EXP = mybir.ActivationFunctionType.Exp

S = 2048          # sequence length
D = 1024          # model dim
HD = 64           # head dim
NHL = 8           # heads per core
HP = 4            # head pairs per core (128 dims each)
DLOC = 512        # d-features per core
ET = D // 128     # 8 contraction tiles over D
ST = S // 128     # 16 s-tiles
QC = S // 512     # 4 query chunks of 512
KK = S // 128     # 16 key tiles of 128
VW = HD + 1       # V block width per head incl. mask column


def build_nc():
    nc = bacc.Bacc(None)
    xT = nc.dram_tensor("xT", [D, S], F32, kind="ExternalInput")
    wqT = nc.dram_tensor("wqT", [D, DLOC], F32, kind="ExternalInput")
    wkT = nc.dram_tensor("wkT", [D, DLOC], F32, kind="ExternalInput")
    wvT = nc.dram_tensor("wvT", [D, DLOC], F32, kind="ExternalInput")
    woT = nc.dram_tensor("woT", [DLOC, D], F32, kind="ExternalInput")
    bq = nc.dram_tensor("bq", [DLOC, 1], F32, kind="ExternalInput")
    bk = nc.dram_tensor("bk", [DLOC, 1], F32, kind="ExternalInput")
    msk = nc.dram_tensor("msk", [S, 1], F32, kind="ExternalInput")
    out = nc.dram_tensor("out", [S, D], F32, kind="ExternalOutput")

    with tile.TileContext(nc) as tc, ExitStack() as ctx:
        res = ctx.enter_context(tc.tile_pool(name="res", bufs=1))

        kt = [res.tile([128, S], BF16, tag=f"kt{i}", name=f"kt{i}") for i in range(HP)]
        vm = [res.tile([128, NHL * VW], BF16, tag=f"vm{i}", name=f"vm{i}") for i in range(KK)]
        valsT = [res.tile([128, S], BF16, tag=f"valsT{i}", name=f"valsT{i}") for i in range(HP)]
        xb = [res.tile([128, S], BF16, tag=f"xb{e}", name=f"xb{e}") for e in range(ET)]
        wq_sb = [res.tile([128, DLOC], BF16, tag=f"wq{e}", name=f"wq{e}") for e in range(ET)]
        wo_sb = [res.tile([128, D], BF16, tag=f"wo{i}", name=f"wo{i}") for i in range(HP)]

        m_sb = res.tile([128, ST], F32, tag="m_sb")
        nc.sync.dma_start(out=m_sb, in_=msk.rearrange("(a p) o -> p (a o)", p=128))
        bq_sb = res.tile([128, HP], F32, tag="bq_sb")
        nc.sync.dma_start(out=bq_sb, in_=bq.rearrange("(a p) o -> p (a o)", p=128))
        bk_sb = res.tile([128, HP], F32, tag="bk_sb")
        nc.sync.dma_start(out=bk_sb, in_=bk.rearrange("(a p) o -> p (a o)", p=128))
        ones8 = res.tile([128, NHL], BF16, tag="ones8")
        nc.vector.memset(ones8, 1.0)

        # current-chunk Q^T tiles, rewritten every q-chunk (double-buffered)
        qtp = ctx.enter_context(tc.tile_pool(name="qtp", bufs=2))

        # ---------- Pass 1: K and V projections (stream xT once) ----------
        with tc.tile_pool(name="pw", bufs=1) as pw, \
             tc.tile_pool(name="wstg", bufs=4) as wstg, \
             tc.tile_pool(name="xstg", bufs=12) as xstg, \
             tc.tile_pool(name="psA", bufs=4, space="PSUM") as psA:
            wk_sb = [pw.tile([128, DLOC], BF16, tag=f"wk{e}", name=f"wk{e}") for e in range(ET)]
            wv_sb = [pw.tile([128, DLOC], BF16, tag=f"wv{e}", name=f"wv{e}") for e in range(ET)]
            for e in range(ET):
                stg = wstg.tile([128, DLOC], F32, tag="wstg", name=f"wkstg{e}")
                nc.sync.dma_start(out=stg, in_=wkT[e * 128:(e + 1) * 128, :])
                nc.scalar.copy(wk_sb[e], stg)
                stg2 = wstg.tile([128, DLOC], F32, tag="wstg", name=f"wvstg{e}")
                nc.sync.dma_start(out=stg2, in_=wvT[e * 128:(e + 1) * 128, :])
                nc.scalar.copy(wv_sb[e], stg2)

            def emit_xchunk(qc):
                cs_x = slice(qc * 512, (qc + 1) * 512)
                for e in range(ET):
                    xs = xstg.tile([128, 512], F32, tag="xstg", name=f"xs{qc}_{e}")
                    nc.sync.dma_start(out=xs, in_=xT[e * 128:(e + 1) * 128, cs_x])
                    nc.scalar.copy(xb[e][:, cs_x], xs)

            emit_xchunk(0)
            for qc in range(QC):
                cs = slice(qc * 512, (qc + 1) * 512)
                if qc + 1 < QC:
                    emit_xchunk(qc + 1)
                if qc == 1:
                    for e in range(ET):
                        stg = wstg.tile([128, DLOC], F32, tag="wstg", name=f"wqstg{e}")
                        nc.sync.dma_start(out=stg, in_=wqT[e * 128:(e + 1) * 128, :])
                        nc.scalar.copy(wq_sb[e], stg)
                if qc == 2:
                    for i in range(HP):
                        stg = wstg.tile([128, D], F32, tag="wostg", name=f"wostg{i}")
                        nc.sync.dma_start(out=stg, in_=woT[i * 128:(i + 1) * 128, :])
                        nc.scalar.copy(wo_sb[i], stg)
                for hp in range(HP):
                    hcols = slice(hp * 128, (hp + 1) * 128)
                    psK = psA.tile([128, 512], F32, tag="psA", name=f"psK{qc}_{hp}")
                    for e in range(ET):
                        nc.tensor.matmul(psK, wk_sb[e][:, hcols], xb[e][:, cs],
                                         start=(e == 0), stop=(e == ET - 1))
                    nc.vector.tensor_scalar_add(kt[hp][:, cs], psK, bk_sb[:, hp:hp + 1])
                for j in range(4):
                    st = qc * 4 + j
                    ss = slice(st * 128, (st + 1) * 128)
                    psV = psA.tile([128, 512], F32, tag="psA", name=f"psV{st}")
                    for e in range(ET):
                        nc.tensor.matmul(psV, xb[e][:, ss], wv_sb[e],
                                         start=(e == 0), stop=(e == ET - 1))
                    mc = m_sb[:, st:st + 1]
                    vmv = vm[st].rearrange("p (h w) -> p h w", w=VW)
                    psVv = psV.rearrange("p (h w) -> p h w", w=HD)
                    nc.vector.tensor_scalar_mul(vmv[:, :, 0:HD], psVv, mc)
                    nc.vector.tensor_scalar_mul(
                        vmv[:, :, HD:VW],
                        ones8.rearrange("p (h o) -> p h o", o=1), mc)

        # ---- Pass 2: pipelined attention + Q projection + out projection ----
        with tc.tile_pool(name="ptp", bufs=24) as ptp, \
             tc.tile_pool(name="sm", bufs=4) as sm, \
             tc.tile_pool(name="ob", bufs=2) as ob, \
             tc.tile_pool(name="psS", bufs=2, space="PSUM") as psSp, \
             tc.tile_pool(name="psO", bufs=2, space="PSUM") as psOp, \
             tc.tile_pool(name="psQ", bufs=1, space="PSUM") as psQp, \
             tc.tile_pool(name="psC", bufs=1, space="PSUM") as psCp:

            # prologue: full Q projection for chunk 0 (alternate PSUM pools)
            qt_next = [qtp.tile([128, 512], BF16, tag=f"qt{g}", name=f"qt0_{g}")
                       for g in range(HP)]
            for g in range(HP):
                pool_g = psQp if g % 2 == 0 else psCp
                psQ = pool_g.tile([128, 512], F32,
                                  tag=("psQ" if g % 2 == 0 else "psC"),
                                  name=f"psQp0_{g}")
                for e in range(ET):
                    nc.tensor.matmul(psQ, wq_sb[e][:, g * 128:(g + 1) * 128],
                                     xb[e][:, 0:512], start=(e == 0), stop=(e == ET - 1))
                nc.vector.tensor_scalar_add(qt_next[g], psQ, bq_sb[:, g:g + 1])

            def make_qproj(qc_next, qt_tiles):
                state = {}
                cs_n = slice(qc_next * 512, (qc_next + 1) * 512)

                def emit(it):
                    g, e = divmod(it, ET)
                    if e == 0:
                        state["psQ"] = psQp.tile([128, 512], F32, tag="psQ",
                                                 name=f"psQ{qc_next}_{g}")
                    nc.tensor.matmul(state["psQ"], wq_sb[e][:, g * 128:(g + 1) * 128],
                                     xb[e][:, cs_n], start=(e == 0), stop=(e == ET - 1))
                    if e == ET - 1:
                        nc.vector.tensor_scalar_add(qt_tiles[g], state["psQ"],
                                                    bq_sb[:, g:g + 1])
                return emit

            def make_outproj(qc_prev):
                state = {}

                def emit(m):
                    grp, hp_i = divmod(m, HP)
                    stl, ec = divmod(grp, 2)
                    st = qc_prev * 4 + stl
                    ss = slice(st * 128, (st + 1) * 128)
                    es = slice(ec * 512, (ec + 1) * 512)
                    if hp_i == 0 and ec == 0:
                        state["ot"] = ob.tile([128, D], F32, tag="ot", name=f"ot{st}")
                    if hp_i == 0:
                        state["psC"] = psCp.tile([128, 512], F32, tag="psC",
                                                 name=f"psC{st}_{ec}")
                    nc.tensor.matmul(state["psC"], valsT[hp_i][:, ss],
                                     wo_sb[hp_i][:, es],
                                     start=(hp_i == 0), stop=(hp_i == HP - 1))
                    if hp_i == HP - 1:
                        nc.vector.tensor_copy(state["ot"][:, es], state["psC"])
                        if ec == 1:
                            nc.sync.dma_start(out=out[ss, :], in_=state["ot"])
                return emit

            def emit_pv(unit, pts_u, psO_pair, kp):
                _, php = unit
                for h2 in range(2):
                    h_prev = php * 2 + h2
                    for u2 in range(2):
                        kk = 2 * kp + u2
                        nc.tensor.matmul(
                            psO_pair[h2][0:VW, :],
                            vm[kk][:, h_prev * VW:(h_prev + 1) * VW],
                            pts_u[h2][kp][:, u2 * 512:(u2 + 1) * 512],
                            start=(kk == 0), stop=(kk == KK - 1))

            def emit_norms(unit, psO_pair):
                uqc, uhp = unit
                ucs = slice(uqc * 512, (uqc + 1) * 512)
                for h2 in range(2):
                    hr = slice(h2 * 64, (h2 + 1) * 64)
                    dn = sm.tile([1, 512], F32, tag="dn", name=f"dn{uqc}_{uhp}_{h2}")
                    nc.vector.tensor_copy(dn, psO_pair[h2][HD:VW, :])
                    nc.vector.reciprocal_approx_fast(out=dn, in_=dn)
                    dnb = sm.tile([64, 512], F32, tag="dnb", name=f"dnb{uqc}_{uhp}_{h2}")
                    nc.gpsimd.partition_broadcast(dnb, dn)
                    nc.vector.tensor_mul(valsT[uhp][hr, ucs], psO_pair[h2][0:HD, :], dnb)

            qproj_emit = None
            outproj_emit = None
            qt_cur = None
            pts_prev = None
            prev_unit = None
            psO_prev = None

            for ui in range(QC * HP):
                qc, hp = divmod(ui, HP)
                if hp == 0:
                    qt_cur = qt_next
                    if qc + 1 < QC:
                        qt_next = [qtp.tile([128, 512], BF16, tag=f"qt{g}",
                                            name=f"qt{qc + 1}_{g}") for g in range(HP)]
                        qproj_emit = make_qproj(qc + 1, qt_next)
                    else:
                        qproj_emit = None
                    outproj_emit = make_outproj(qc - 1) if qc > 0 else None

                pts_cur = [[None] * 8 for _ in range(2)]
                if prev_unit is not None:
                    psO_prev = [psOp.tile([128, 512], F32, tag="psO",
                                          name=f"psO{ui}_{h2}") for h2 in range(2)]

                for kp in range(8):
                    it = hp * 8 + kp
                    psS_pair = [psSp.tile([128, 1024], F32, tag="psS",
                                          name=f"psS{ui}_{kp}_{h2}") for h2 in range(2)]
                    for h2 in range(2):
                        hr = slice(h2 * 64, (h2 + 1) * 64)
                        for u2 in range(2):
                            kk = 2 * kp + u2
                            ks = slice(kk * 128, (kk + 1) * 128)
                            nc.tensor.matmul(psS_pair[h2][:, u2 * 512:(u2 + 1) * 512],
                                             kt[hp][hr, ks], qt_cur[hp][hr, :],
                                             start=True, stop=True)
                        pt = ptp.tile([128, 1024], BF16, tag="pt",
                                      name=f"pt{ui}_{kp}_{h2}")
                        nc.scalar.activation(pt, psS_pair[h2], EXP, scale=0.125)
                        pts_cur[h2][kp] = pt
                        # PV for the previous unit, same kp, matching head half
                        if prev_unit is not None:
                            _, php = prev_unit
                            h_prev = php * 2 + h2
                            for u2 in range(2):
                                kk = 2 * kp + u2
                                nc.tensor.matmul(
                                    psO_prev[h2][0:VW, :],
                                    vm[kk][:, h_prev * VW:(h_prev + 1) * VW],
                                    pts_prev[h2][kp][:, u2 * 512:(u2 + 1) * 512],
                                    start=(kk == 0), stop=(kk == KK - 1))
                    if qproj_emit is not None:
                        qproj_emit(it)
                    if outproj_emit is not None and 8 <= it < 24:
                        m = (it - 8) * 2
                        outproj_emit(m)
                        outproj_emit(m + 1)
                    if kp == 7 and prev_unit is not None:
                        emit_norms(prev_unit, psO_prev)

                pts_prev = pts_cur
                prev_unit = (qc, hp)

            # epilogue: PV + norm for the last unit, then out-proj for qc=3
            psO_last = [psOp.tile([128, 512], F32, tag="psO", name=f"psOL_{h2}")
                        for h2 in range(2)]
            for kp in range(8):
                emit_pv(prev_unit, pts_prev, psO_last, kp)
            emit_norms(prev_unit, psO_last)
            outproj_emit = make_outproj(QC - 1)
            for m in range(32):
                outproj_emit(m)

    nc.finalize()
    return nc


_NC_CACHE = None


def _get_nc():
    global _NC_CACHE
    if _NC_CACHE is None:
        _NC_CACHE = build_nc()
    return _NC_CACHE


def make_in_maps(x, mask, Wq, bq, Wk, bk, Wv, Wo):
    in_maps = []
    for c in range(8):
        b = c // 2
        dsl = slice((c % 2) * DLOC, (c % 2) * DLOC + DLOC)
        in_maps.append({
            "xT": np.ascontiguousarray(x[b].T, dtype=np.float32),
            "wqT": np.ascontiguousarray(Wq[dsl, :].T, dtype=np.float32),
            "wkT": np.ascontiguousarray(Wk[dsl, :].T, dtype=np.float32),
            "wvT": np.ascontiguousarray(Wv[dsl, :].T, dtype=np.float32),
            "woT": np.ascontiguousarray(Wo[:, dsl].T, dtype=np.float32),
            "bq": np.ascontiguousarray(bq[dsl], dtype=np.float32)[:, None],
            "bk": np.ascontiguousarray(bk[dsl], dtype=np.float32)[:, None],
            "msk": mask[b].astype(np.float32)[:, None],
        })
    return in_maps


def assemble(results, Wo, bo, bv):
    out = np.empty((4, S, D), dtype=np.float32)
    for b in range(4):
        out[b] = results[2 * b]["out"] + results[2 * b + 1]["out"]
    out += (bo + bv @ Wo.T).astype(np.float32)
    return out


def run(x, mask, Wq, bq, Wk, bk, Wv, bv, Wo, bo, trace=False):
    nc = _get_nc()
    in_maps = make_in_maps(x, mask, Wq, bq, Wk, bk, Wv, Wo)
    res = run_bass_kernel_spmd(nc, in_maps, list(range(8)), trace=trace)
    return assemble(res.results, Wo, bo, bv), res


def kernel(x, mask, Wq, bq, Wk, bk, Wv, bv, Wo, bo):
    out, _ = run(x, mask, Wq, bq, Wk, bk, Wv, bv, Wo, bo)
    return out
